# revision 11
# baseline (speedup 1.0000x reference)
"""nn_LEAStereo cost-volume + 3D-conv + bilinear upsample on 8 TRN2 NeuronCores.

Shapes (hardcoded per spec): x_feat/y_feat [2,3,32,88,116] f32,
w_match [1,64,3,3,3] f32. Output [2,33,260,346] f32.

Algorithm: only frame t=2 of each batch survives the [:, 2:] frame drop, so
2 frames matter. The cost volume slot d holds (x masked to w>=d, y shifted
by d); contracting channels first (tap images Xt = wl^T x, Yt = wr^T y,
27 taps each) collapses the 3x3x3 conv over the volume to 2D structure:

  cost[d,h,w] = F[h,w]*[w-d>=2] + G[w-d][h,w]  (w-d in -2..1)   (left half)
              + R[h,w-d] - edge corrections                      (right half)

F/G/R are [88,~150] images built from shifted sums of the tap images; the
d-loop reduces to a few batched strided vector ops over [88, 33*116].
Bilinear resize = row-interp matmul (88->65 rows per core) + exact x3
column upsample (W-1=115, OW-1=345 -> phase weights 0,1/3,2/3).

Sharding: 8 cores = 2 frames x 4 row-quarters of the 260 output rows.
Identical SPMD program; per-core behavior differs only through input data
(frame tensors + per-quarter row-interp matrix).
"""
import os
import numpy as np
import ml_dtypes

BF16 = ml_dtypes.bfloat16

C = 32
D = 33
H, W = 88, 116
HP, WP = 92, 118           # padded grid (row h+1, col w+1; rows 90,91 zero)
FLAT = HP * WP             # 10856
QROWS = 23                 # HP/4 rows per phase-A quarter
QF = QROWS * WP            # 2714
OH, OW = 260, 346
OHC = OH // 4              # 65 output rows per core
NTAP = 27

_BUILT = {}


def _taps(kd, kh, kw):
    return kd * 9 + kh * 3 + kw


def _row_matrix():
    ys = np.linspace(0.0, H - 1.0, OH)
    y0 = np.floor(ys).astype(np.int32)
    y1 = np.minimum(y0 + 1, H - 1)
    wy = (ys - y0).astype(np.float32)
    Rt = np.zeros((H, OH), dtype=np.float32)
    for j in range(OH):
        Rt[y0[j], j] += 1.0 - wy[j]
        Rt[y1[j], j] += wy[j]
    return Rt


def _build_nc():
    import concourse.bacc as bacc
    import concourse.mybir as mybir
    import bass_rust
    from concourse.tile import TileContext

    dt = mybir.dt
    Alu = mybir.AluOpType
    ActF = mybir.ActivationFunctionType

    nc = bacc.Bacc("TRN2", target_bir_lowering=False, debug=False)

    xp = nc.dram_tensor("xp", [C, FLAT], dt.bfloat16, kind="ExternalInput")
    yp = nc.dram_tensor("yp", [C, FLAT], dt.bfloat16, kind="ExternalInput")
    wl = nc.dram_tensor("wl", [C, 32], dt.bfloat16, kind="ExternalInput")
    wr = nc.dram_tensor("wr", [C, 32], dt.bfloat16, kind="ExternalInput")
    rt = nc.dram_tensor("rt", [H, OHC], dt.bfloat16, kind="ExternalInput")
    outd = nc.dram_tensor("out", [D, OHC, OW], dt.float32, kind="ExternalOutput")
    xtd = nc.dram_tensor("xtd", [NTAP, FLAT], dt.bfloat16)
    ytd = nc.dram_tensor("ytd", [NTAP, FLAT], dt.bfloat16)

    def diag_ap(tile_ap, offset, d_step, d_cnt, w_step, w_cnt):
        """Custom overlapping AP on a 2-D SBUF tile: [part, d_cnt, w_cnt]."""
        a = tile_ap.copy()
        a.ap = bass_rust.VecI64Pair(
            [tuple(a.ap[0]), (d_step, d_cnt), (w_step, w_cnt)])
        a.offset = a.offset + offset
        return a

    with TileContext(nc) as tc:
        with (
            tc.tile_pool(name="io", bufs=1) as io,
            tc.tile_pool(name="psA", bufs=3, space="PSUM") as psA,
            tc.tile_pool(name="stage", bufs=1) as stage,
            tc.tile_pool(name="img", bufs=1) as imgp,
            tc.tile_pool(name="psR", bufs=3, space="PSUM") as psR,
        ):
            xs = io.tile([C, FLAT], dt.bfloat16)
            ys = io.tile([C, FLAT], dt.bfloat16)
            wls = io.tile([C, 32], dt.bfloat16)
            wrs = io.tile([C, 32], dt.bfloat16)
            rts = io.tile([H, OHC], dt.bfloat16)
            nc.sync.dma_start(out=xs[:, :], in_=xp[:, :])
            nc.sync.dma_start(out=ys[:, :], in_=yp[:, :])
            nc.sync.dma_start(out=wls[:, :], in_=wl[:, :])
            nc.sync.dma_start(out=wrs[:, :], in_=wr[:, :])
            nc.sync.dma_start(out=rts[:, :], in_=rt[:, :])

            # ---- Phase A: tap GEMMs, 4 h-quarters stacked on psum partitions
            CH = [(i * 512, 512) for i in range(5)] + [(2560, QF - 2560)]
            xt4 = stage.tile([128, QF], dt.bfloat16)
            yt4 = stage.tile([128, QF], dt.bfloat16)
            for src, wt, dst4, dram in ((xs, wls, xt4, xtd),
                                        (ys, wrs, yt4, ytd)):
                for off, nn in CH:
                    ps = psA.tile([128, 512], dt.float32, tag="psA")
                    for q in range(4):
                        nc.tensor.matmul(
                            ps[q * 32:(q + 1) * 32, :nn],
                            wt[:, :],
                            src[:, q * QF + off: q * QF + off + nn],
                            start=True, stop=True,
                            tile_position=(0, q * 32))
                    nc.scalar.activation(dst4[:, off:off + nn], ps[:, :nn],
                                         ActF.Copy)
                for q in range(4):
                    nc.sync.dma_start(
                        out=dram[:, q * QF:(q + 1) * QF],
                        in_=dst4[q * 32:q * 32 + NTAP, :])

            # ---- transpose gather: [27,(h,w)] -> [92(h), 27, 118], then two
            # partition-shifted copies (compute engines need start partition
            # 0/32/64/96, so the kh shift must not live on the partition dim).
            TW = NTAP * WP
            xtT = stage.tile([HP, TW], dt.bfloat16)
            ytT = stage.tile([HP, TW], dt.bfloat16)
            nc.sync.dma_start(
                out=xtT.rearrange("p (t w) -> p t w", w=WP),
                in_=xtd.rearrange("t (h w) -> h t w", w=WP))
            nc.sync.dma_start(
                out=ytT.rearrange("p (t w) -> p t w", w=WP),
                in_=ytd.rearrange("t (h w) -> h t w", w=WP))
            XKH = [xtT]
            YKH = [ytT]
            for kh in (1, 2):
                xk = stage.tile([H, TW], dt.bfloat16, name=f"xk{kh}")
                yk = stage.tile([H, TW], dt.bfloat16, name=f"yk{kh}")
                nc.sync.dma_start(out=xk[:, :], in_=xtT[kh:kh + H, :])
                nc.sync.dma_start(out=yk[:, :], in_=ytT[kh:kh + H, :])
                XKH.append(xk)
                YKH.append(yk)

            def xterm(kd, kh, kw):
                t = _taps(kd, kh, kw)
                v = XKH[kh].rearrange("p (t w) -> p t w", w=WP)
                return v[0:H, t, kw:kw + W]

            def yterm_u(kd, kh, kw, umin, n):
                """Y tap slice for u-range [umin, umin+n): grid col u+kw-kd+1."""
                t = _taps(kd, kh, kw)
                c0 = umin + kw - kd + 1
                v = YKH[kh].rearrange("p (t w) -> p t w", w=WP)
                return v[0:H, t, c0:c0 + n]

            # ---- Phase B: images.  Left on vector, right on gpsimd.
            def acc(eng, dst, terms):
                eng.tensor_copy(out=dst, in_=terms[0])
                for t in terms[1:]:
                    eng.tensor_tensor(out=dst, in0=dst, in1=t, op=Alu.add)

            V, G_ = nc.vector, nc.gpsimd
            A0 = imgp.tile([H, W], dt.float32)
            A1 = imgp.tile([H, W], dt.float32)
            A2 = imgp.tile([H, W], dt.float32)
            Fi = imgp.tile([H, W], dt.float32)
            F0 = imgp.tile([H, W], dt.float32)
            F32 = imgp.tile([H, W], dt.float32)
            Gm2 = imgp.tile([H, W], dt.float32)
            Gm1 = imgp.tile([H, W], dt.float32)
            G0 = imgp.tile([H, W], dt.float32)
            G1 = imgp.tile([H, W], dt.float32)
            P20 = imgp.tile([H, W], dt.float32)
            P21 = imgp.tile([H, W], dt.float32)
            for kd, Ai in ((0, A0), (1, A1), (2, A2)):
                acc(V, Ai[:, :], [xterm(kd, kh, kw)
                                  for kh in range(3) for kw in range(3)])
            V.tensor_tensor(out=F32[:, :], in0=A0[:, :], in1=A1[:, :], op=Alu.add)
            V.tensor_tensor(out=F0[:, :], in0=A1[:, :], in1=A2[:, :], op=Alu.add)
            V.tensor_tensor(out=Fi[:, :], in0=F32[:, :], in1=A2[:, :], op=Alu.add)
            # P_kd[t'] term sets: kw >= kd - t'
            acc(V, Gm2[:, :], [xterm(0, kh, 2) for kh in range(3)])
            acc(V, Gm1[:, :], [xterm(0, kh, kw) for kh in range(3)
                               for kw in (1, 2)]
                + [xterm(1, kh, 2) for kh in range(3)])
            acc(V, P20[:, :], [xterm(2, kh, 2) for kh in range(3)])
            acc(V, P21[:, :], [xterm(2, kh, kw) for kh in range(3)
                               for kw in (1, 2)])
            # G0 = A0 + P1[0](kw>=1) + P20
            V.tensor_tensor(out=G0[:, :], in0=A0[:, :], in1=P20[:, :], op=Alu.add)
            for kh in range(3):
                for kw in (1, 2):
                    V.tensor_tensor(out=G0[:, :], in0=G0[:, :],
                                    in1=xterm(1, kh, kw), op=Alu.add)
            V.tensor_tensor(out=G1[:, :], in0=F32[:, :], in1=P21[:, :], op=Alu.add)
            # edge columns
            Gcol0 = imgp.tile([H, 2], dt.float32)    # d=0: w=0,1
            Gcol32 = imgp.tile([H, 4], dt.float32)   # d=32: w=30..33
            V.tensor_tensor(out=Gcol0[:, 0:1], in0=G0[:, 0:1], in1=A0[:, 0:1],
                            op=Alu.subtract)
            V.tensor_tensor(out=Gcol0[:, 1:2], in0=G1[:, 1:2], in1=A0[:, 1:2],
                            op=Alu.subtract)
            V.tensor_copy(out=Gcol32[:, 0:1], in_=Gm2[:, 30:31])
            V.tensor_copy(out=Gcol32[:, 1:2], in_=Gm1[:, 31:32])
            V.tensor_tensor(out=Gcol32[:, 2:3], in0=G0[:, 32:33],
                            in1=P20[:, 32:33], op=Alu.subtract)
            V.tensor_tensor(out=Gcol32[:, 3:4], in0=G1[:, 33:34],
                            in1=P21[:, 33:34], op=Alu.subtract)

            # Right-half images [88, 150]: col = u + 32, u in [-2, 115]
            RK = [imgp.tile([H, 150], dt.float32, name=f"rk{i}")
                  for i in range(3)]
            Ri = imgp.tile([H, 150], dt.float32)
            R0 = imgp.tile([H, 150], dt.float32)
            R32 = imgp.tile([H, 150], dt.float32)
            Rcorr = imgp.tile([H, 150], dt.float32)
            for kd in range(3):
                G_.memset(RK[kd][:, :], 0.0)
                for kh in range(3):
                    for kw in range(3):
                        umin = max(-2, kd - kw)
                        umax = 114 if (kw - kd) == 2 else 115
                        n = umax - umin + 1
                        dst = RK[kd][:, 32 + umin:32 + umin + n]
                        G_.tensor_tensor(out=dst, in0=dst,
                                         in1=yterm_u(kd, kh, kw, umin, n),
                                         op=Alu.add)
            G_.tensor_tensor(out=R32[:, :], in0=RK[0][:, :], in1=RK[1][:, :],
                             op=Alu.add)
            G_.tensor_tensor(out=R0[:, :], in0=RK[1][:, :], in1=RK[2][:, :],
                             op=Alu.add)
            G_.tensor_tensor(out=Ri[:, :], in0=R32[:, :], in1=RK[2][:, :],
                             op=Alu.add)
            G_.memset(Rcorr[:, :], 0.0)
            for kd in range(3):
                for kh in range(3):
                    umin = max(-2, kd - 2)
                    umax = 114 if kd == 0 else 115
                    n = umax - umin + 1
                    dst = Rcorr[:, 32 + umin:32 + umin + n]
                    G_.tensor_tensor(out=dst, in0=dst,
                                     in1=yterm_u(kd, kh, 2, umin, n),
                                     op=Alu.add)
            def ycol(kd, kh, col):
                v = YKH[kh].rearrange("p (t w) -> p t w", w=WP)
                return v[0:H, _taps(kd, kh, 2), col:col + 1]

            Rc0 = imgp.tile([H, 1], dt.float32)   # d=0 (u=115): kd in {1,2}
            Rc32 = imgp.tile([H, 1], dt.float32)  # d=32 (u=83): kd in {0,1}
            acc(G_, Rc0[:, :], [ycol(kd, kh, 118 - kd)
                                for kd in (1, 2) for kh in range(3)])
            acc(G_, Rc32[:, :], [ycol(kd, kh, 86 - kd)
                                 for kd in (0, 1) for kh in range(3)])

            # ---- Assembly: cost [88, 33*116] bf16
            cost = stage.tile([H, D * W], dt.bfloat16)
            costv = cost.rearrange("p (d w) -> p d w", w=W)
            # 1. F select over all d (w - d - 2 >= 0 keep else 0)
            G_.affine_select(
                out=costv[:, :, :],
                in_=Fi[:, :].unsqueeze(1).broadcast_to((H, D, W)),
                pattern=[[-1, D], [1, W]], base=-2,
                compare_op=Alu.is_ge, fill=0.0, channel_multiplier=0)
            # 2. G diagonal writes (interior d)
            for tp, dlo, img in ((-2, 2, Gm2), (-1, 1, Gm1), (0, 1, G0),
                                 (1, 1, G1)):
                cnt = 31 - dlo + 1
                s = dlo * 117 + tp
                V.tensor_copy(out=cost[:, s:s + 117 * cnt:117],
                              in_=img[:, dlo + tp:dlo + tp + cnt])
            # 3. R diagonal add over all d: cost[:,d,w] += R[:, 32+w-d]
            V.tensor_tensor(out=costv[:, :, :], in0=costv[:, :, :],
                            in1=diag_ap(Ri[:, :], 32, -1, D, 1, W), op=Alu.add)
            # 4. right-edge corr (interior d): cost[:,d,115] -= Rcorr[:,147-d]
            V.tensor_tensor(out=cost[:, 231:231 + 116 * 31:116],
                            in0=cost[:, 231:231 + 116 * 31:116],
                            in1=Rcorr[:, 146:115:-1], op=Alu.subtract)
            # 5. fixup d=0
            G_.affine_select(out=costv[:, 0, :], in_=F0[:, :],
                             pattern=[[1, W]], base=-2,
                             compare_op=Alu.is_ge, fill=0.0,
                             channel_multiplier=0)
            V.tensor_copy(out=cost[:, 0:2], in_=Gcol0[:, :])
            V.tensor_tensor(out=costv[:, 0, :], in0=costv[:, 0, :],
                            in1=R0[:, 32:148], op=Alu.add)
            V.tensor_tensor(out=cost[:, 115:116], in0=cost[:, 115:116],
                            in1=Rc0[:, :], op=Alu.subtract)
            # 6. fixup d=32
            G_.affine_select(out=costv[:, 32, :], in_=F32[:, :],
                             pattern=[[1, W]], base=-34,
                             compare_op=Alu.is_ge, fill=0.0,
                             channel_multiplier=0)
            V.tensor_copy(out=cost[:, 32 * 116 + 30:32 * 116 + 34],
                          in_=Gcol32[:, :])
            V.tensor_tensor(out=costv[:, 32, :], in0=costv[:, 32, :],
                            in1=R32[:, 0:116], op=Alu.add)
            V.tensor_tensor(out=cost[:, 32 * 116 + 115:32 * 116 + 116],
                            in0=cost[:, 32 * 116 + 115:32 * 116 + 116],
                            in1=Rc32[:, :], op=Alu.subtract)

            # ---- Resize: row matmul + exact x3 column phases
            u = stage.tile([OHC, D * W], dt.bfloat16)
            v = stage.tile([OHC, D * W], dt.bfloat16)
            RCH = [(i * 512, 512) for i in range(7)] + [(3584, D * W - 3584)]
            for off, nn in RCH:
                ps = psR.tile([OHC, 512], dt.float32, tag="psR")
                nc.tensor.matmul(ps[:, :nn], rts[:, :], cost[:, off:off + nn],
                                 start=True, stop=True)
                nc.scalar.activation(u[:, off:off + nn], ps[:, :nn], ActF.Copy,
                                     scale=1.0 / 3.0)
                nc.scalar.activation(v[:, off:off + nn], ps[:, :nn], ActF.Copy,
                                     scale=2.0 / 3.0)
            outsb = stage.tile([OHC, D * OW], dt.float32)
            ov = outsb.rearrange("p (d w) -> p d w", w=OW)
            uvv = u.rearrange("p (d w) -> p d w", w=W)
            vvv = v.rearrange("p (d w) -> p d w", w=W)
            V.tensor_tensor(out=ov[:, :, 0:OW:3], in0=uvv[:, :, :],
                            in1=vvv[:, :, :], op=Alu.add)
            V.tensor_tensor(out=ov[:, :, 1:OW:3], in0=vvv[:, :, 0:115],
                            in1=uvv[:, :, 1:116], op=Alu.add)
            G_.tensor_tensor(out=ov[:, :, 2:OW:3], in0=uvv[:, :, 0:115],
                             in1=vvv[:, :, 1:116], op=Alu.add)
            nc.sync.dma_start(out=outd.rearrange("d j w -> j d w"),
                              in_=ov[:, :, :])
    nc.compile()
    return nc


def _prep_inputs(x_feat, y_feat, w_match):
    """Host-side shard prep: per-core input dicts."""
    x_feat = np.asarray(x_feat, dtype=np.float32)
    y_feat = np.asarray(y_feat, dtype=np.float32)
    w_match = np.asarray(w_match, dtype=np.float32)
    wl = np.zeros((C, 32), dtype=BF16)
    wr = np.zeros((C, 32), dtype=BF16)
    wl[:, :NTAP] = w_match[0, :C].reshape(C, NTAP)
    wr[:, :NTAP] = w_match[0, C:].reshape(C, NTAP)
    Rt = _row_matrix()
    in_maps = []
    for core in range(8):
        n, q = divmod(core, 4)
        xpad = np.zeros((C, HP, WP), dtype=BF16)
        ypad = np.zeros((C, HP, WP), dtype=BF16)
        xpad[:, 1:89, 1:117] = x_feat[n, 2]
        ypad[:, 1:89, 1:117] = y_feat[n, 2]
        in_maps.append({
            "xp": xpad.reshape(C, FLAT),
            "yp": ypad.reshape(C, FLAT),
            "wl": wl, "wr": wr,
            "rt": Rt[:, q * OHC:(q + 1) * OHC].astype(BF16),
        })
    return in_maps


def kernel(x_feat, y_feat, w_match):
    from concourse.bass_utils import run_bass_kernel_spmd

    if "nc" not in _BUILT:
        _BUILT["nc"] = _build_nc()
    nc = _BUILT["nc"]
    in_maps = _prep_inputs(x_feat, y_feat, w_match)
    trace = bool(int(os.environ.get("KERNEL_TRACE", "0")))
    res = run_bass_kernel_spmd(
        nc, in_maps, core_ids=list(range(8)),
        trace=trace,
        trace_cores=list(range(8)) if trace else None,
    )
    _BUILT["last_result"] = res
    out = np.empty((2, D, OH, OW), dtype=np.float32)
    for core in range(8):
        n, q = divmod(core, 4)
        out[n, :, q * OHC:(q + 1) * OHC, :] = res.results[core]["out"]
    return out


# revision 13
# speedup vs baseline: 1.0192x; 1.0192x over previous
"""nn_LEAStereo cost-volume + 3D-conv + bilinear upsample on 8 TRN2 NeuronCores.

Shapes (hardcoded per spec): x_feat/y_feat [2,3,32,88,116] f32,
w_match [1,64,3,3,3] f32. Output [2,33,260,346] f32.

Algorithm: only frame t=2 of each batch survives the [:, 2:] frame drop, so
2 frames matter. Contracting channels first (tap images Xt = wl^T x,
Yt = wr^T y, 27 taps each) collapses the masked/shifted cost volume + 3x3x3
conv to 2D structure:

  cost[d,h,w] = F[h,w]*[w-d>=2] + G[w-d][h,w]  (w-d in -2..1)   (left half)
              + R[h,w-d] - right-edge corrections                (right half)

F/G/R are [88,~150] images built from shifted sums of the tap images; the
d-loop reduces to a few batched strided vector ops over [88, 33*116].
Bilinear resize = row-interp matmul (88->65 rows per core) + exact x3
column upsample. The three column phases (weights 0, 1/3, 2/3) are kept
contiguous on-chip and interleaved on the host (strided SBUF writes are
slow on DVE).

Sharding: 8 cores = 2 frames x 4 row-quarters of the 260 output rows.
Identical SPMD program; per-core behavior differs only through input data
(frame tensors + per-quarter row-interp matrix).
"""
import os
import numpy as np
import ml_dtypes

BF16 = ml_dtypes.bfloat16

C = 32
D = 33
H, W = 88, 116
HP, WP = 92, 118           # padded grid (row h+1, col w+1; rows 90,91 zero)
FLAT = HP * WP             # 10856
QROWS = 23                 # HP/4 rows per phase-A quarter
QF = QROWS * WP            # 2714
OH, OW = 260, 346
OHC = OH // 4              # 65 output rows per core
NTAP = 27

_BUILT = {}


def _taps(kd, kh, kw):
    return kd * 9 + kh * 3 + kw


def _row_matrix():
    ys = np.linspace(0.0, H - 1.0, OH)
    y0 = np.floor(ys).astype(np.int32)
    y1 = np.minimum(y0 + 1, H - 1)
    wy = (ys - y0).astype(np.float32)
    Rt = np.zeros((H, OH), dtype=np.float32)
    for j in range(OH):
        Rt[y0[j], j] += 1.0 - wy[j]
        Rt[y1[j], j] += wy[j]
    return Rt


def _build_nc():
    import concourse.bacc as bacc
    import concourse.mybir as mybir
    import bass_rust
    from concourse.tile import TileContext

    dt = mybir.dt
    Alu = mybir.AluOpType
    ActF = mybir.ActivationFunctionType

    nc = bacc.Bacc("TRN2", target_bir_lowering=False, debug=False)

    xp = nc.dram_tensor("xp", [C, FLAT], dt.bfloat16, kind="ExternalInput")
    yp = nc.dram_tensor("yp", [C, FLAT], dt.bfloat16, kind="ExternalInput")
    wl = nc.dram_tensor("wl", [C, 32], dt.bfloat16, kind="ExternalInput")
    wr = nc.dram_tensor("wr", [C, 32], dt.bfloat16, kind="ExternalInput")
    rt = nc.dram_tensor("rt", [H, OHC], dt.bfloat16, kind="ExternalInput")
    # phase-major output: [phase, d, row, m]; host interleaves columns
    outd = nc.dram_tensor("out", [3, D, OHC, W], dt.float32,
                          kind="ExternalOutput")
    xtd = nc.dram_tensor("xtd", [NTAP, FLAT], dt.bfloat16)
    ytd = nc.dram_tensor("ytd", [NTAP, FLAT], dt.bfloat16)

    def diag_ap(tile_ap, offset, dims):
        """Custom (possibly overlapping) AP on a 2-D SBUF tile."""
        a = tile_ap.copy()
        a.ap = bass_rust.VecI64Pair([tuple(a.ap[0])] + list(dims))
        a.offset = a.offset + offset
        return a

    with TileContext(nc) as tc:
        with (
            tc.tile_pool(name="io", bufs=1) as io,
            tc.tile_pool(name="feed", bufs=4) as feed,
            tc.tile_pool(name="psA", bufs=3, space="PSUM") as psA,
            tc.tile_pool(name="stage", bufs=1) as stage,
            tc.tile_pool(name="img", bufs=1) as imgp,
            tc.tile_pool(name="psR", bufs=3, space="PSUM") as psR,
        ):
            A = nc.scalar     # ACT engine: copies/casts/scaled copies (+ 2nd DMA ring)
            V = nc.vector
            G_ = nc.gpsimd
            S = nc.sync

            wls = io.tile([C, 32], dt.bfloat16)
            wrs = io.tile([C, 32], dt.bfloat16)
            rts = io.tile([H, OHC], dt.bfloat16)
            S.dma_start(out=wls[:, :], in_=wl[:, :])
            A.dma_start(out=wrs[:, :], in_=wr[:, :])
            A.dma_start(out=rts[:, :], in_=rt[:, :])

            # ---- Phase A: tap GEMMs, 4 h-quarters stacked on psum partitions.
            # Inputs streamed per quarter so PE starts early.
            CH = [(i * 512, 512) for i in range(5)] + [(2560, QF - 2560)]
            xt4 = stage.tile([128, QF], dt.bfloat16)
            yt4 = stage.tile([128, QF], dt.bfloat16)
            for src_d, wt, dst4, dram, ring in ((xp, wls, xt4, xtd, S),
                                               (yp, wrs, yt4, ytd, A)):
                qtiles = []
                for q in range(4):
                    qt = feed.tile([C, QF], dt.bfloat16, tag="feed")
                    ring.dma_start(out=qt[:, :],
                                   in_=src_d[:, q * QF:(q + 1) * QF])
                    qtiles.append(qt)
                for off, nn in CH:
                    ps = psA.tile([128, 512], dt.float32, tag="psA")
                    for q in range(4):
                        nc.tensor.matmul(
                            ps[q * 32:(q + 1) * 32, :nn],
                            wt[:, :],
                            qtiles[q][:, off:off + nn],
                            start=True, stop=True,
                            tile_position=(0, q * 32))
                    A.activation(dst4[:, off:off + nn], ps[:, :nn], ActF.Copy)
                for q in range(4):
                    ring.dma_start(out=dram[:, q * QF:(q + 1) * QF],
                                   in_=dst4[q * 32:q * 32 + NTAP, :])

            # ---- transpose gather: [27,(h,w)] -> [92(h), 27, 118], then two
            # partition-shifted copies (compute engines need start partition
            # 0/32/64/96, so the kh shift must not live on the partition dim).
            TW = NTAP * WP
            xtT = stage.tile([HP, TW], dt.bfloat16)
            ytT = stage.tile([HP, TW], dt.bfloat16)
            HT = 13  # tap-split for parallel gathers on the two rings
            for dram, dstT, r1, r2 in ((xtd, xtT, S, A), (ytd, ytT, A, S)):
                dv = dstT.rearrange("p (t w) -> p t w", w=WP)
                sv = dram.rearrange("t (h w) -> h t w", w=WP)
                r1.dma_start(out=dv[:, :HT, :], in_=sv[:, :HT, :])
                r2.dma_start(out=dv[:, HT:, :], in_=sv[:, HT:, :])
            XKH = [xtT]
            YKH = [ytT]
            for kh in (1, 2):
                xk = stage.tile([H, TW], dt.bfloat16, name=f"xk{kh}")
                yk = stage.tile([H, TW], dt.bfloat16, name=f"yk{kh}")
                S.dma_start(out=xk[:, :], in_=xtT[kh:kh + H, :])
                A.dma_start(out=yk[:, :], in_=ytT[kh:kh + H, :])
                XKH.append(xk)
                YKH.append(yk)

            def xterm(kd, kh, kw):
                t = _taps(kd, kh, kw)
                v = XKH[kh].rearrange("p (t w) -> p t w", w=WP)
                return v[0:H, t, kw:kw + W]

            def yterm_u(kd, kh, kw, umin, n):
                """Y tap slice for u-range [umin, umin+n): grid col u+kw-kd+1."""
                t = _taps(kd, kh, kw)
                c0 = umin + kw - kd + 1
                v = YKH[kh].rearrange("p (t w) -> p t w", w=WP)
                return v[0:H, t, c0:c0 + n]

            # ---- Phase B images. Left on vector, right on gpsimd; initial
            # copies (casts) on ACT.
            def acc(eng, dst, terms):
                A.activation(dst, terms[0], ActF.Copy)
                for t in terms[1:]:
                    eng.tensor_tensor(out=dst, in0=dst, in1=t, op=Alu.add)

            A0 = imgp.tile([H, W], dt.float32)
            A1 = imgp.tile([H, W], dt.float32)
            A2 = imgp.tile([H, W], dt.float32)
            Fi = imgp.tile([H, W], dt.float32)
            F0 = imgp.tile([H, W], dt.float32)
            F32 = imgp.tile([H, W], dt.float32)
            Gm2 = imgp.tile([H, W], dt.float32)
            Gm1 = imgp.tile([H, W], dt.float32)
            G0 = imgp.tile([H, W], dt.float32)
            G1 = imgp.tile([H, W], dt.float32)
            P20 = imgp.tile([H, W], dt.float32)
            P21 = imgp.tile([H, W], dt.float32)
            for kd, Ai in ((0, A0), (1, A1), (2, A2)):
                acc(V, Ai[:, :], [xterm(kd, kh, kw)
                                  for kh in range(3) for kw in range(3)])
            V.tensor_tensor(out=F32[:, :], in0=A0[:, :], in1=A1[:, :], op=Alu.add)
            V.tensor_tensor(out=F0[:, :], in0=A1[:, :], in1=A2[:, :], op=Alu.add)
            V.tensor_tensor(out=Fi[:, :], in0=F32[:, :], in1=A2[:, :], op=Alu.add)
            # P_kd[t'] term sets: kw >= kd - t'
            acc(V, Gm2[:, :], [xterm(0, kh, 2) for kh in range(3)])
            acc(V, Gm1[:, :], [xterm(0, kh, kw) for kh in range(3)
                               for kw in (1, 2)]
                + [xterm(1, kh, 2) for kh in range(3)])
            acc(V, P20[:, :], [xterm(2, kh, 2) for kh in range(3)])
            acc(V, P21[:, :], [xterm(2, kh, kw) for kh in range(3)
                               for kw in (1, 2)])
            V.tensor_tensor(out=G0[:, :], in0=A0[:, :], in1=P20[:, :], op=Alu.add)
            for kh in range(3):
                for kw in (1, 2):
                    V.tensor_tensor(out=G0[:, :], in0=G0[:, :],
                                    in1=xterm(1, kh, kw), op=Alu.add)
            V.tensor_tensor(out=G1[:, :], in0=F32[:, :], in1=P21[:, :], op=Alu.add)
            # edge columns
            Gcol0 = imgp.tile([H, 2], dt.float32)    # d=0: w=0,1
            Gcol32 = imgp.tile([H, 4], dt.float32)   # d=32: w=30..33
            V.tensor_tensor(out=Gcol0[:, 0:1], in0=G0[:, 0:1], in1=A0[:, 0:1],
                            op=Alu.subtract)
            V.tensor_tensor(out=Gcol0[:, 1:2], in0=G1[:, 1:2], in1=A0[:, 1:2],
                            op=Alu.subtract)
            A.activation(Gcol32[:, 0:1], Gm2[:, 30:31], ActF.Copy)
            A.activation(Gcol32[:, 1:2], Gm1[:, 31:32], ActF.Copy)
            V.tensor_tensor(out=Gcol32[:, 2:3], in0=G0[:, 32:33],
                            in1=P20[:, 32:33], op=Alu.subtract)
            V.tensor_tensor(out=Gcol32[:, 3:4], in0=G1[:, 33:34],
                            in1=P21[:, 33:34], op=Alu.subtract)

            # Right-half images [88, 150]: col = u + 32, u in [-2, 115]
            RK = [imgp.tile([H, 150], dt.float32, name=f"rk{i}")
                  for i in range(3)]
            Ri = imgp.tile([H, 150], dt.float32)
            R0 = imgp.tile([H, 150], dt.float32)
            R32 = imgp.tile([H, 150], dt.float32)
            Rcorr = imgp.tile([H, 150], dt.float32)
            for kd in range(3):
                G_.memset(RK[kd][:, :], 0.0)
                for kh in range(3):
                    for kw in range(3):
                        umin = max(-2, kd - kw)
                        umax = 114 if (kw - kd) == 2 else 115
                        n = umax - umin + 1
                        dst = RK[kd][:, 32 + umin:32 + umin + n]
                        G_.tensor_tensor(out=dst, in0=dst,
                                         in1=yterm_u(kd, kh, kw, umin, n),
                                         op=Alu.add)
            G_.tensor_tensor(out=R32[:, :], in0=RK[0][:, :], in1=RK[1][:, :],
                             op=Alu.add)
            G_.tensor_tensor(out=R0[:, :], in0=RK[1][:, :], in1=RK[2][:, :],
                             op=Alu.add)
            G_.tensor_tensor(out=Ri[:, :], in0=R32[:, :], in1=RK[2][:, :],
                             op=Alu.add)
            G_.memset(Rcorr[:, :], 0.0)
            for kd in range(3):
                for kh in range(3):
                    umin = max(-2, kd - 2)
                    umax = 114 if kd == 0 else 115
                    n = umax - umin + 1
                    dst = Rcorr[:, 32 + umin:32 + umin + n]
                    G_.tensor_tensor(out=dst, in0=dst,
                                     in1=yterm_u(kd, kh, 2, umin, n),
                                     op=Alu.add)

            def ycol(kd, kh, col):
                v = YKH[kh].rearrange("p (t w) -> p t w", w=WP)
                return v[0:H, _taps(kd, kh, 2), col:col + 1]

            Rc0 = imgp.tile([H, 1], dt.float32)   # d=0 (u=115): kd in {1,2}
            Rc32 = imgp.tile([H, 1], dt.float32)  # d=32 (u=83): kd in {0,1}
            acc(G_, Rc0[:, :], [ycol(kd, kh, 118 - kd)
                                for kd in (1, 2) for kh in range(3)])
            acc(G_, Rc32[:, :], [ycol(kd, kh, 86 - kd)
                                 for kd in (0, 1) for kh in range(3)])

            # ---- Assembly: cost [88, 33*116] bf16
            cost = stage.tile([H, D * W], dt.bfloat16)
            costv = cost.rearrange("p (d w) -> p d w", w=W)
            # 1. F select over all d (w - d - 2 >= 0 keep else 0)
            G_.affine_select(
                out=costv[:, :, :],
                in_=Fi[:, :].unsqueeze(1).broadcast_to((H, D, W)),
                pattern=[[-1, D], [1, W]], base=-2,
                compare_op=Alu.is_ge, fill=0.0, channel_multiplier=0)
            # 2. G diagonal writes (interior d), on ACT (1-input copies)
            for tp, dlo, img in ((-2, 2, Gm2), (-1, 1, Gm1), (0, 1, G0),
                                 (1, 1, G1)):
                cnt = 31 - dlo + 1
                s = dlo * 117 + tp
                A.activation(cost[:, s:s + 117 * cnt:117],
                             img[:, dlo + tp:dlo + tp + cnt], ActF.Copy)
            # 3. R diagonal add over all d: cost[:,d,w] += R[:, 32+w-d]
            #    split d-halves across V and G_.
            DSPL = 17
            V.tensor_tensor(
                out=costv[:, :DSPL, :], in0=costv[:, :DSPL, :],
                in1=diag_ap(Ri[:, :], 32, [(-1, DSPL), (1, W)]), op=Alu.add)
            G_.tensor_tensor(
                out=costv[:, DSPL:, :], in0=costv[:, DSPL:, :],
                in1=diag_ap(Ri[:, :], 32 - DSPL, [(-1, D - DSPL), (1, W)]),
                op=Alu.add)
            # 4. right-edge corr (interior d): cost[:,d,115] -= Rcorr[:,147-d]
            V.tensor_tensor(out=cost[:, 231:231 + 116 * 31:116],
                            in0=cost[:, 231:231 + 116 * 31:116],
                            in1=Rcorr[:, 146:115:-1], op=Alu.subtract)
            # 5. fixup d=0
            G_.affine_select(out=costv[:, 0, :], in_=F0[:, :],
                             pattern=[[1, W]], base=-2,
                             compare_op=Alu.is_ge, fill=0.0,
                             channel_multiplier=0)
            V.tensor_copy(out=cost[:, 0:2], in_=Gcol0[:, :])
            V.tensor_tensor(out=costv[:, 0, :], in0=costv[:, 0, :],
                            in1=R0[:, 32:148], op=Alu.add)
            V.tensor_tensor(out=cost[:, 115:116], in0=cost[:, 115:116],
                            in1=Rc0[:, :], op=Alu.subtract)
            # 6. fixup d=32
            G_.affine_select(out=costv[:, 32, :], in_=F32[:, :],
                             pattern=[[1, W]], base=-34,
                             compare_op=Alu.is_ge, fill=0.0,
                             channel_multiplier=0)
            V.tensor_copy(out=cost[:, 32 * 116 + 30:32 * 116 + 34],
                          in_=Gcol32[:, :])
            V.tensor_tensor(out=costv[:, 32, :], in0=costv[:, 32, :],
                            in1=R32[:, 0:116], op=Alu.add)
            V.tensor_tensor(out=cost[:, 32 * 116 + 115:32 * 116 + 116],
                            in0=cost[:, 32 * 116 + 115:32 * 116 + 116],
                            in1=Rc32[:, :], op=Alu.subtract)

            # ---- Resize: row matmul; column phases contiguous (p0,p1,p2),
            # interleaved on the host.
            u = stage.tile([OHC, D * W], dt.bfloat16)
            v = stage.tile([OHC, D * W], dt.bfloat16)
            p0 = stage.tile([OHC, D * W], dt.float32)
            p1 = stage.tile([OHC, D * W], dt.float32)
            p2 = stage.tile([OHC, D * W], dt.float32)
            RCH = [(i * 512, 512) for i in range(7)] + [(3584, D * W - 3584)]
            for off, nn in RCH:
                ps = psR.tile([OHC, 512], dt.float32, tag="psR")
                nc.tensor.matmul(ps[:, :nn], rts[:, :], cost[:, off:off + nn],
                                 start=True, stop=True)
                A.activation(u[:, off:off + nn], ps[:, :nn], ActF.Copy,
                             scale=1.0 / 3.0)
                A.activation(v[:, off:off + nn], ps[:, :nn], ActF.Copy,
                             scale=2.0 / 3.0)
            uvv = u.rearrange("p (d w) -> p d w", w=W)
            vvv = v.rearrange("p (d w) -> p d w", w=W)
            p1v = p1.rearrange("p (d w) -> p d w", w=W)
            p2v = p2.rearrange("p (d w) -> p d w", w=W)
            # p0 = u + v (= r1); p1[m] = v[m] + u[m+1]; p2[m] = u[m] + v[m+1]
            V.tensor_tensor(out=p0[:, :], in0=u[:, :], in1=v[:, :], op=Alu.add)
            V.tensor_tensor(out=p1v[:, :, 0:115], in0=vvv[:, :, 0:115],
                            in1=uvv[:, :, 1:116], op=Alu.add)
            G_.tensor_tensor(out=p2v[:, :, 0:115], in0=uvv[:, :, 0:115],
                             in1=vvv[:, :, 1:116], op=Alu.add)
            ph_dma = (S, A, S)
            for r, (pt, ring) in enumerate(zip((p0, p1, p2), ph_dma)):
                wlim = W if r == 0 else W - 1
                ring.dma_start(
                    out=outd[r].rearrange("d j w -> j d w")[:, :, 0:wlim],
                    in_=pt.rearrange("p (d w) -> p d w", w=W)[:, :, 0:wlim])
    nc.compile()
    return nc


def _prep_inputs(x_feat, y_feat, w_match):
    """Host-side shard prep: per-core input dicts."""
    x_feat = np.asarray(x_feat, dtype=np.float32)
    y_feat = np.asarray(y_feat, dtype=np.float32)
    w_match = np.asarray(w_match, dtype=np.float32)
    wl = np.zeros((C, 32), dtype=BF16)
    wr = np.zeros((C, 32), dtype=BF16)
    wl[:, :NTAP] = w_match[0, :C].reshape(C, NTAP)
    wr[:, :NTAP] = w_match[0, C:].reshape(C, NTAP)
    Rt = _row_matrix()
    in_maps = []
    for core in range(8):
        n, q = divmod(core, 4)
        xpad = np.zeros((C, HP, WP), dtype=BF16)
        ypad = np.zeros((C, HP, WP), dtype=BF16)
        xpad[:, 1:89, 1:117] = x_feat[n, 2]
        ypad[:, 1:89, 1:117] = y_feat[n, 2]
        in_maps.append({
            "xp": xpad.reshape(C, FLAT),
            "yp": ypad.reshape(C, FLAT),
            "wl": wl, "wr": wr,
            "rt": Rt[:, q * OHC:(q + 1) * OHC].astype(BF16),
        })
    return in_maps


def _interleave(out_slice, ph):
    """ph: [3, 33, 65, 116] phase-major -> out_slice [33, 65, 346]."""
    out_slice[:, :, 0::3] = ph[0]
    out_slice[:, :, 1::3] = ph[1][:, :, :115]
    out_slice[:, :, 2::3] = ph[2][:, :, :115]


def kernel(x_feat, y_feat, w_match):
    from concourse.bass_utils import run_bass_kernel_spmd

    if "nc" not in _BUILT:
        _BUILT["nc"] = _build_nc()
    nc = _BUILT["nc"]
    in_maps = _prep_inputs(x_feat, y_feat, w_match)
    trace = bool(int(os.environ.get("KERNEL_TRACE", "0")))
    res = run_bass_kernel_spmd(
        nc, in_maps, core_ids=list(range(8)),
        trace=trace,
        trace_cores=list(range(8)) if trace else None,
    )
    _BUILT["last_result"] = res
    out = np.empty((2, D, OH, OW), dtype=np.float32)
    for core in range(8):
        n, q = divmod(core, 4)
        _interleave(out[n, :, q * OHC:(q + 1) * OHC, :],
                    res.results[core]["out"])
    return out


# revision 17
# speedup vs baseline: 1.0945x; 1.0738x over previous
"""nn_LEAStereo cost-volume + 3D-conv + bilinear upsample on 8 TRN2 NeuronCores.

Shapes (hardcoded per spec): x_feat/y_feat [2,3,32,88,116] f32,
w_match [1,64,3,3,3] f32. Output [2,33,260,346] f32.

Algorithm: only frame t=2 of each batch survives the [:, 2:] frame drop, so
2 frames matter. Contracting channels first (tap images Xt = wl^T x,
Yt = wr^T y, 27 taps each) collapses the masked/shifted cost volume + 3x3x3
conv to 2D structure:

  cost[d,h,w] = F[h,w]*[w-d>=2] + G[w-d][h,w]  (w-d in -2..1)   (left half)
              + R[h,w-d] - right-edge corrections                (right half)

F/G/R are [88,~150] images built from shifted sums of the tap images,
batched 3-wide over kd (taps kd*9+kh*3+kw are 9 apart, so one strided op
accumulates all three kd blocks at once). The d-loop reduces to a few
batched strided vector ops over [88, 33*116]. Bilinear resize = row-interp
matmul (88->65 rows per core) + exact x3 column upsample; the three column
phases (weights 0, 1/3, 2/3) stay contiguous on-chip and are interleaved on
the host.

Sharding: 8 cores = 2 frames x 4 row-quarters of the 260 output rows.
Identical SPMD program; per-core behavior differs only through input data
(frame tensors + per-quarter row-interp matrix).
"""
import os
import numpy as np
import ml_dtypes

BF16 = ml_dtypes.bfloat16

C = 32
D = 33
H, W = 88, 116
HP, WP = 92, 118           # padded grid (row h+1, col w+1; rows 90,91 zero)
WB = 124                   # tap-block width: grid col w' lives at w'+3
FLAT = HP * WP             # 10856
QROWS = 23                 # HP/4 rows per phase-A quarter
QF = QROWS * WP            # 2714
OH, OW = 260, 346
OHC = OH // 4              # 65 output rows per core
NTAP = 27
TWB = NTAP * WB            # 3348

_BUILT = {}


def _row_matrix():
    ys = np.linspace(0.0, H - 1.0, OH)
    y0 = np.floor(ys).astype(np.int32)
    y1 = np.minimum(y0 + 1, H - 1)
    wy = (ys - y0).astype(np.float32)
    Rt = np.zeros((H, OH), dtype=np.float32)
    for j in range(OH):
        Rt[y0[j], j] += 1.0 - wy[j]
        Rt[y1[j], j] += wy[j]
    return Rt


def _build_nc():
    import concourse.bacc as bacc
    import concourse.mybir as mybir
    import bass_rust
    from concourse.tile import TileContext

    dt = mybir.dt
    Alu = mybir.AluOpType
    ActF = mybir.ActivationFunctionType

    nc = bacc.Bacc("TRN2", target_bir_lowering=False, debug=False)

    xp = nc.dram_tensor("xp", [C, FLAT], dt.bfloat16, kind="ExternalInput")
    yp = nc.dram_tensor("yp", [C, FLAT], dt.bfloat16, kind="ExternalInput")
    wl = nc.dram_tensor("wl", [C, 32], dt.bfloat16, kind="ExternalInput")
    wr = nc.dram_tensor("wr", [C, 32], dt.bfloat16, kind="ExternalInput")
    rt = nc.dram_tensor("rt", [H, OHC], dt.bfloat16, kind="ExternalInput")
    # phase-major output: [phase, d, row, m]; host interleaves columns
    outd = nc.dram_tensor("out", [3, D, OHC, W], dt.float32,
                          kind="ExternalOutput")
    # h-major tap images in DRAM: [92, 27*124] (write side pays the small
    # descriptors, overlapped with phase-A compute; read side is contiguous)
    xtd = nc.dram_tensor("xtd", [HP, TWB], dt.bfloat16)
    ytd = nc.dram_tensor("ytd", [HP, TWB], dt.bfloat16)

    def strided(tile_ap, offset, dims):
        """Custom (possibly overlapping) AP on a 2-D SBUF tile."""
        a = tile_ap.copy()
        a.ap = bass_rust.VecI64Pair([tuple(a.ap[0])] + list(dims))
        a.offset = a.offset + offset
        return a

    with TileContext(nc) as tc:
        with (
            tc.tile_pool(name="io", bufs=1) as io,
            tc.tile_pool(name="feed", bufs=4) as feed,
            tc.tile_pool(name="psA", bufs=1, space="PSUM") as psA,
            tc.tile_pool(name="stage", bufs=1) as stage,
            tc.tile_pool(name="img", bufs=1) as imgp,
            tc.tile_pool(name="psR", bufs=2, space="PSUM") as psR,
        ):
            A = nc.scalar     # ACT engine (+ 2nd HWDGE DMA ring)
            V = nc.vector
            G_ = nc.gpsimd
            S = nc.sync

            wls = io.tile([C, 32], dt.bfloat16)
            wrs = io.tile([C, 32], dt.bfloat16)
            rts = io.tile([H, OHC], dt.bfloat16)
            S.dma_start(out=wls[:, :], in_=wl[:, :])
            A.dma_start(out=wrs[:, :], in_=wr[:, :])
            A.dma_start(out=rts[:, :], in_=rt[:, :])

            # ---- Phase A: tap GEMMs, 4 h-quarters stacked on psum
            # partitions; quarters streamed so PE starts after the first
            # feed DMA. Chunks are whole h-rows so the copyback lands in the
            # 124-wide block layout.
            RCHUNK = [(0, 4), (4, 4), (8, 4), (12, 4), (16, 4), (20, 3)]
            xt4 = stage.tile([128, QROWS * WB], dt.bfloat16)
            yt4 = stage.tile([128, QROWS * WB], dt.bfloat16)
            # zero the 3+3 pad columns of every 124-block
            for t4, eng in ((xt4, V), (yt4, G_)):
                v4 = t4.rearrange("p (r b) -> p r b", b=WB)
                eng.memset(v4[:, :, 0:3], 0.0)
                eng.memset(v4[:, :, 121:124], 0.0)

            psq = [psA.tile([128, 512], dt.float32, name=f"psq{i}", tag=f"ps{i}")
                   for i in range(6)]
            for src_d, wt, dst4, dram, ring in ((xp, wls, xt4, xtd, S),
                                               (yp, wrs, yt4, ytd, A)):
                qtiles = []
                for q in range(4):
                    qt = feed.tile([C, QF], dt.bfloat16, tag="feed")
                    ring.dma_start(out=qt[:, :],
                                   in_=src_d[:, q * QF:(q + 1) * QF])
                    qtiles.append(qt)
                for q in range(4):
                    for ci, (r0, nr) in enumerate(RCHUNK):
                        nn = nr * WP
                        nc.tensor.matmul(
                            psq[ci][q * 32:(q + 1) * 32, :nn],
                            wt[:, :],
                            qtiles[q][:, r0 * WP:r0 * WP + nn],
                            start=True, stop=True,
                            tile_position=(0, q * 32))
                # copybacks after all 4 quarters hit a chunk's psum
                d4 = dst4.rearrange("p (r b) -> p r b", b=WB)
                for ci, (r0, nr) in enumerate(RCHUNK):
                    A.activation(
                        d4[:, r0:r0 + nr, 3:121],
                        psq[ci][:, :nr * WP].rearrange(
                            "p (r w) -> p r w", w=WP),
                        ActF.Copy)
                # write-out: h-major scatter (small descriptors, overlapped)
                vd = dram.rearrange("h (t b) -> t h b", b=WB)
                for q in range(4):
                    rng = ring if q % 2 == 0 else (A if ring is S else S)
                    rng.dma_start(
                        out=vd[:, q * QROWS:(q + 1) * QROWS, :],
                        in_=dst4[q * 32:q * 32 + NTAP, :].rearrange(
                            "p (r b) -> p r b", b=WB))

            # ---- contiguous read-back + two partition-shifted copies
            # (compute engines need start partition 0/32/64/96).
            xtT = stage.tile([HP, TWB], dt.bfloat16)
            ytT = stage.tile([HP, TWB], dt.bfloat16)
            S.dma_start(out=xtT[:, :], in_=xtd[:, :])
            A.dma_start(out=ytT[:, :], in_=ytd[:, :])
            XKH = [xtT]
            YKH = [ytT]
            for kh in (1, 2):
                xk = stage.tile([H, TWB], dt.bfloat16, name=f"xk{kh}")
                yk = stage.tile([H, TWB], dt.bfloat16, name=f"yk{kh}")
                S.dma_start(out=xk[:, :], in_=xtT[kh:kh + H, :])
                A.dma_start(out=yk[:, :], in_=ytT[kh:kh + H, :])
                XKH.append(xk)
                YKH.append(yk)

            def xg(kh, kw):
                """Left term, kd-grouped: [88, 3, 116] (blocks kd=0,1,2)."""
                v = XKH[kh].rearrange("p (t b) -> p t b", b=WB)
                t0 = kh * 3 + kw
                return v[0:H, t0:t0 + 19:9, kw + 3:kw + 119]

            def yg(kh, kw):
                """Right term, kd-grouped, u in [-2,115]: [88, 3, 118].
                Block col = u + kw - kd + 5; kd-block stride = 9*WB - 1."""
                t0 = kh * 3 + kw
                base = t0 * WB + (kw + 2)   # kd=0, u=-2: grid col u+kw+1 -> +3
                return strided(YKH[kh][0:H, :], base,
                               [(9 * WB - 1, 3), (1, 118)])

            # ---- Phase B images, kd-grouped.
            def accw(eng, dst, terms, init_act=True):
                if init_act:
                    A.activation(dst, terms[0], ActF.Copy)
                    rest = terms[1:]
                else:
                    rest = terms
                for t in rest:
                    eng.tensor_tensor(out=dst, in0=dst, in1=t, op=Alu.add)

            # S_kd[j] = sum over kh, kw>=j of term(kd,kh,kw): [88, 3(kd), 116]
            SA0 = imgp.tile([H, 3 * W], dt.float32)
            SA1 = imgp.tile([H, 3 * W], dt.float32)
            SA2 = imgp.tile([H, 3 * W], dt.float32)
            sa0 = SA0.rearrange("p (k w) -> p k w", w=W)
            sa1 = SA1.rearrange("p (k w) -> p k w", w=W)
            sa2 = SA2.rearrange("p (k w) -> p k w", w=W)
            accw(V, sa0[:, :, :], [xg(kh, kw) for kh in range(3)
                                   for kw in range(3)])
            accw(V, sa1[:, :, :], [xg(kh, kw) for kh in range(3)
                                   for kw in (1, 2)])
            accw(V, sa2[:, :, :], [xg(kh, 2) for kh in range(3)])
            # combos; aliases: Gm2 = SA2.k0, P20 = SA2.k2, P21 = SA1.k2
            Fi = imgp.tile([H, W], dt.float32)
            F0 = imgp.tile([H, W], dt.float32)
            F32 = imgp.tile([H, W], dt.float32)
            Gm1 = imgp.tile([H, W], dt.float32)
            G0 = imgp.tile([H, W], dt.float32)
            G1 = imgp.tile([H, W], dt.float32)
            V.tensor_tensor(out=F32[:, :], in0=sa0[:, 0, :], in1=sa0[:, 1, :],
                            op=Alu.add)
            V.tensor_tensor(out=F0[:, :], in0=sa0[:, 1, :], in1=sa0[:, 2, :],
                            op=Alu.add)
            V.tensor_tensor(out=Fi[:, :], in0=F32[:, :], in1=sa0[:, 2, :],
                            op=Alu.add)
            V.tensor_tensor(out=Gm1[:, :], in0=sa1[:, 0, :], in1=sa2[:, 1, :],
                            op=Alu.add)
            V.tensor_tensor(out=G0[:, :], in0=sa0[:, 0, :], in1=sa1[:, 1, :],
                            op=Alu.add)
            V.tensor_tensor(out=G0[:, :], in0=G0[:, :], in1=sa2[:, 2, :],
                            op=Alu.add)
            V.tensor_tensor(out=G1[:, :], in0=F32[:, :], in1=sa1[:, 2, :],
                            op=Alu.add)
            Gm2 = sa2[:, 0, :]
            P20 = sa2[:, 2, :]
            P21 = sa1[:, 2, :]
            # edge columns
            Gcol0 = imgp.tile([H, 2], dt.float32)    # d=0: w=0,1
            Gcol32 = imgp.tile([H, 4], dt.float32)   # d=32: w=30..33
            V.tensor_tensor(out=Gcol0[:, 0:1], in0=G0[:, 0:1],
                            in1=sa0[:, 0, 0:1], op=Alu.subtract)
            V.tensor_tensor(out=Gcol0[:, 1:2], in0=G1[:, 1:2],
                            in1=sa0[:, 0, 1:2], op=Alu.subtract)
            A.activation(Gcol32[:, 0:1], Gm2[:, 30:31], ActF.Copy)
            A.activation(Gcol32[:, 1:2], Gm1[:, 31:32], ActF.Copy)
            V.tensor_tensor(out=Gcol32[:, 2:3], in0=G0[:, 32:33],
                            in1=P20[:, 32:33], op=Alu.subtract)
            V.tensor_tensor(out=Gcol32[:, 3:4], in0=G1[:, 33:34],
                            in1=P21[:, 33:34], op=Alu.subtract)

            # Right-half images: RKc [88, 3(kd), 150], col = u+32, u in
            # [-2,115] -> cols 30..147; cols 0..29,148..149 stay zero.
            RKc = imgp.tile([H, 3 * 150], dt.float32)
            RCc = imgp.tile([H, 3 * 150], dt.float32)
            rkc = RKc.rearrange("p (k u) -> p k u", u=150)
            rcc = RCc.rearrange("p (k u) -> p k u", u=150)
            G_.memset(RKc[:, :], 0.0)
            G_.memset(RCc[:, :], 0.0)
            for kh in range(3):
                for kw in range(3):
                    G_.tensor_tensor(out=rkc[:, :, 30:148],
                                     in0=rkc[:, :, 30:148],
                                     in1=yg(kh, kw), op=Alu.add)
                G_.tensor_tensor(out=rcc[:, :, 30:148],
                                 in0=rcc[:, :, 30:148],
                                 in1=yg(kh, 2), op=Alu.add)
            Ri = imgp.tile([H, 150], dt.float32)
            R0 = imgp.tile([H, 150], dt.float32)
            R32 = imgp.tile([H, 150], dt.float32)
            Rcorr = imgp.tile([H, 150], dt.float32)
            G_.tensor_tensor(out=R32[:, :], in0=rkc[:, 0, :], in1=rkc[:, 1, :],
                             op=Alu.add)
            G_.tensor_tensor(out=R0[:, :], in0=rkc[:, 1, :], in1=rkc[:, 2, :],
                             op=Alu.add)
            G_.tensor_tensor(out=Ri[:, :], in0=R32[:, :], in1=rkc[:, 2, :],
                             op=Alu.add)
            G_.tensor_tensor(out=Rcorr[:, :], in0=rcc[:, 0, :],
                             in1=rcc[:, 1, :], op=Alu.add)
            G_.tensor_tensor(out=Rcorr[:, :], in0=Rcorr[:, :],
                             in1=rcc[:, 2, :], op=Alu.add)
            Rc0 = imgp.tile([H, 1], dt.float32)   # d=0 (u=115): kd in {1,2}
            Rc32 = imgp.tile([H, 1], dt.float32)  # d=32 (u=83): kd in {0,1}
            G_.tensor_tensor(out=Rc0[:, :], in0=rcc[:, 1, 147:148],
                             in1=rcc[:, 2, 147:148], op=Alu.add)
            G_.tensor_tensor(out=Rc32[:, :], in0=rcc[:, 0, 115:116],
                             in1=rcc[:, 1, 115:116], op=Alu.add)

            # ---- Assembly: cost [88, 33*116] bf16
            cost = stage.tile([H, D * W], dt.bfloat16)
            costv = cost.rearrange("p (d w) -> p d w", w=W)
            # 1. F select over all d (w - d - 2 >= 0 keep else 0)
            G_.affine_select(
                out=costv[:, :, :],
                in_=Fi[:, :].unsqueeze(1).broadcast_to((H, D, W)),
                pattern=[[-1, D], [1, W]], base=-2,
                compare_op=Alu.is_ge, fill=0.0, channel_multiplier=0)
            # 2. G diagonal writes (interior d), on ACT (1-input copies)
            for tp, dlo, img in ((-2, 2, Gm2), (-1, 1, Gm1[:, :]),
                                 (0, 1, G0[:, :]), (1, 1, G1[:, :])):
                cnt = 31 - dlo + 1
                s = dlo * 117 + tp
                A.activation(cost[:, s:s + 117 * cnt:117],
                             img[:, dlo + tp:dlo + tp + cnt], ActF.Copy)
            # 3. R diagonal add over all d: cost[:,d,w] += R[:, 32+w-d]
            DSPL = 17
            V.tensor_tensor(
                out=costv[:, :DSPL, :], in0=costv[:, :DSPL, :],
                in1=strided(Ri[:, :], 32, [(-1, DSPL), (1, W)]), op=Alu.add)
            G_.tensor_tensor(
                out=costv[:, DSPL:, :], in0=costv[:, DSPL:, :],
                in1=strided(Ri[:, :], 32 - DSPL, [(-1, D - DSPL), (1, W)]),
                op=Alu.add)
            # 4. right-edge corr (interior d): cost[:,d,115] -= Rcorr[:,147-d]
            V.tensor_tensor(out=cost[:, 231:231 + 116 * 31:116],
                            in0=cost[:, 231:231 + 116 * 31:116],
                            in1=Rcorr[:, 146:115:-1], op=Alu.subtract)
            # 5. fixup d=0
            G_.affine_select(out=costv[:, 0, :], in_=F0[:, :],
                             pattern=[[1, W]], base=-2,
                             compare_op=Alu.is_ge, fill=0.0,
                             channel_multiplier=0)
            V.tensor_copy(out=cost[:, 0:2], in_=Gcol0[:, :])
            V.tensor_tensor(out=costv[:, 0, :], in0=costv[:, 0, :],
                            in1=R0[:, 32:148], op=Alu.add)
            V.tensor_tensor(out=cost[:, 115:116], in0=cost[:, 115:116],
                            in1=Rc0[:, :], op=Alu.subtract)
            # 6. fixup d=32
            G_.affine_select(out=costv[:, 32, :], in_=F32[:, :],
                             pattern=[[1, W]], base=-34,
                             compare_op=Alu.is_ge, fill=0.0,
                             channel_multiplier=0)
            V.tensor_copy(out=cost[:, 32 * 116 + 30:32 * 116 + 34],
                          in_=Gcol32[:, :])
            V.tensor_tensor(out=costv[:, 32, :], in0=costv[:, 32, :],
                            in1=R32[:, 0:116], op=Alu.add)
            V.tensor_tensor(out=cost[:, 32 * 116 + 115:32 * 116 + 116],
                            in0=cost[:, 32 * 116 + 115:32 * 116 + 116],
                            in1=Rc32[:, :], op=Alu.subtract)

            # ---- Resize: row matmul; column phases contiguous (p0,p1,p2),
            # interleaved on the host. Work split in two d-halves so the
            # first half's output DMAs overlap the second half's compute.
            u = stage.tile([OHC, D * W], dt.bfloat16)
            v = stage.tile([OHC, D * W], dt.bfloat16)
            p0 = stage.tile([OHC, D * W], dt.float32)
            p1 = stage.tile([OHC, D * W], dt.float32)
            p2 = stage.tile([OHC, D * W], dt.float32)
            uvv = u.rearrange("p (d w) -> p d w", w=W)
            vvv = v.rearrange("p (d w) -> p d w", w=W)
            p1v = p1.rearrange("p (d w) -> p d w", w=W)
            p2v = p2.rearrange("p (d w) -> p d w", w=W)
            HALVES = [(0, DSPL, (0, 512), (512, 512), (1024, 512),
                       (1536, 460)),
                      (DSPL, D, (1972, 512), (2484, 512), (2996, 512),
                       (3508, 320))]
            for dlo, dhi, *chunks in HALVES:
                for off, nn in chunks:
                    ps = psR.tile([OHC, 512], dt.float32, tag="psR")
                    nc.tensor.matmul(ps[:, :nn], rts[:, :],
                                     cost[:, off:off + nn],
                                     start=True, stop=True)
                    A.activation(u[:, off:off + nn], ps[:, :nn], ActF.Copy,
                                 scale=1.0 / 3.0)
                    A.activation(v[:, off:off + nn], ps[:, :nn], ActF.Copy,
                                 scale=2.0 / 3.0)
                c0, c1 = dlo * W, dhi * W
                # p0 = u + v (= r1); p1[m] = v[m]+u[m+1]; p2[m] = u[m]+v[m+1]
                V.tensor_tensor(out=p0[:, c0:c1], in0=u[:, c0:c1],
                                in1=v[:, c0:c1], op=Alu.add)
                V.tensor_tensor(out=p1v[:, dlo:dhi, 0:115],
                                in0=vvv[:, dlo:dhi, 0:115],
                                in1=uvv[:, dlo:dhi, 1:116], op=Alu.add)
                G_.tensor_tensor(out=p2v[:, dlo:dhi, 0:115],
                                 in0=uvv[:, dlo:dhi, 0:115],
                                 in1=vvv[:, dlo:dhi, 1:116], op=Alu.add)
                for r, (pt, ring) in enumerate(((p0, S), (p1, A), (p2, S))):
                    wlim = W if r == 0 else W - 1
                    ring.dma_start(
                        out=outd[r, dlo:dhi].rearrange(
                            "d j w -> j d w")[:, :, 0:wlim],
                        in_=pt.rearrange(
                            "p (d w) -> p d w", w=W)[:, dlo:dhi, 0:wlim])
    nc.compile()
    return nc


def _prep_inputs(x_feat, y_feat, w_match):
    """Host-side shard prep: per-core input dicts."""
    x_feat = np.asarray(x_feat, dtype=np.float32)
    y_feat = np.asarray(y_feat, dtype=np.float32)
    w_match = np.asarray(w_match, dtype=np.float32)
    wl = np.zeros((C, 32), dtype=BF16)
    wr = np.zeros((C, 32), dtype=BF16)
    wl[:, :NTAP] = w_match[0, :C].reshape(C, NTAP)
    wr[:, :NTAP] = w_match[0, C:].reshape(C, NTAP)
    Rt = _row_matrix()
    in_maps = []
    for core in range(8):
        n, q = divmod(core, 4)
        xpad = np.zeros((C, HP, WP), dtype=BF16)
        ypad = np.zeros((C, HP, WP), dtype=BF16)
        xpad[:, 1:89, 1:117] = x_feat[n, 2]
        ypad[:, 1:89, 1:117] = y_feat[n, 2]
        in_maps.append({
            "xp": xpad.reshape(C, FLAT),
            "yp": ypad.reshape(C, FLAT),
            "wl": wl, "wr": wr,
            "rt": Rt[:, q * OHC:(q + 1) * OHC].astype(BF16),
        })
    return in_maps


def _interleave(out_slice, ph):
    """ph: [3, 33, 65, 116] phase-major -> out_slice [33, 65, 346]."""
    out_slice[:, :, 0::3] = ph[0]
    out_slice[:, :, 1::3] = ph[1][:, :, :115]
    out_slice[:, :, 2::3] = ph[2][:, :, :115]


def kernel(x_feat, y_feat, w_match):
    from concourse.bass_utils import run_bass_kernel_spmd

    if "nc" not in _BUILT:
        _BUILT["nc"] = _build_nc()
    nc = _BUILT["nc"]
    in_maps = _prep_inputs(x_feat, y_feat, w_match)
    trace = bool(int(os.environ.get("KERNEL_TRACE", "0")))
    res = run_bass_kernel_spmd(
        nc, in_maps, core_ids=list(range(8)),
        trace=trace,
        trace_cores=list(range(8)) if trace else None,
    )
    _BUILT["last_result"] = res
    out = np.empty((2, D, OH, OW), dtype=np.float32)
    for core in range(8):
        n, q = divmod(core, 4)
        _interleave(out[n, :, q * OHC:(q + 1) * OHC, :],
                    res.results[core]["out"])
    return out


# revision 20
# speedup vs baseline: 1.2326x; 1.1262x over previous
"""nn_LEAStereo cost-volume + 3D-conv + bilinear upsample on 8 TRN2 NeuronCores.

Shapes (hardcoded per spec): x_feat/y_feat [2,3,32,88,116] f32,
w_match [1,64,3,3,3] f32. Output [2,33,260,346] f32.

Algorithm: only frame t=2 of each batch survives the [:, 2:] frame drop, so
2 frames matter. Contracting channels first (tap images Xt = wl^T x,
Yt = wr^T y, 27 taps each) collapses the masked/shifted cost volume + 3x3x3
conv to 2D structure:

  cost[d,h,w] = F[h,w]*[w-d>=2] + G[w-d][h,w]  (w-d in -2..1)   (left half)
              + R[h,w-d] - right-edge corrections                (right half)

F/G/R are [88,~150] images built from shifted sums of the tap images,
batched 3-wide over kd (taps kd*9+kh*3+kw are 9 apart, so one strided op
accumulates all three kd blocks at once). The d-loop reduces to a few
batched strided vector ops over [88, 33*116]. Bilinear resize = row-interp
matmul (88->65 rows per core) + exact x3 column upsample; the three column
phases (weights 0, 1/3, 2/3) stay contiguous on-chip and are interleaved on
the host.

Sharding: 8 cores = 2 frames x 4 row-quarters of the 260 output rows.
Identical SPMD program; per-core behavior differs only through input data
(frame tensors + per-quarter row-interp matrix).
"""
import os
import numpy as np
import ml_dtypes

BF16 = ml_dtypes.bfloat16

C = 32
D = 33
H, W = 88, 116
HP, WP = 92, 118           # padded grid (row h+1, col w+1; rows 90,91 zero)
WB = 124                   # tap-block width: grid col w' lives at w'+3
FLAT = HP * WP             # 10856
QROWS = 23                 # HP/4 rows per phase-A quarter
QF = QROWS * WP            # 2714
OH, OW = 260, 346
OHC = OH // 4              # 65 output rows per core
NTAP = 27
TWB = NTAP * WB            # 3348

_BUILT = {}


def _row_matrix():
    ys = np.linspace(0.0, H - 1.0, OH)
    y0 = np.floor(ys).astype(np.int32)
    y1 = np.minimum(y0 + 1, H - 1)
    wy = (ys - y0).astype(np.float32)
    Rt = np.zeros((H, OH), dtype=np.float32)
    for j in range(OH):
        Rt[y0[j], j] += 1.0 - wy[j]
        Rt[y1[j], j] += wy[j]
    return Rt


def _build_nc():
    import concourse.bacc as bacc
    import concourse.mybir as mybir
    import bass_rust
    from concourse.tile import TileContext

    dt = mybir.dt
    Alu = mybir.AluOpType
    ActF = mybir.ActivationFunctionType

    nc = bacc.Bacc("TRN2", target_bir_lowering=False, debug=False)

    xp = nc.dram_tensor("xp", [C, FLAT], dt.bfloat16, kind="ExternalInput")
    yp = nc.dram_tensor("yp", [C, FLAT], dt.bfloat16, kind="ExternalInput")
    wl = nc.dram_tensor("wl", [C, 32], dt.bfloat16, kind="ExternalInput")
    wr = nc.dram_tensor("wr", [C, 32], dt.bfloat16, kind="ExternalInput")
    rt = nc.dram_tensor("rt", [H, OHC], dt.bfloat16, kind="ExternalInput")
    # phase-major output: [phase, d, row, m]; host interleaves columns
    outd = nc.dram_tensor("out", [3, D, OHC, W], dt.float32,
                          kind="ExternalOutput")
    # h-major tap images in DRAM: [92, 27*124] (write side pays the small
    # descriptors, overlapped with phase-A compute; read side is contiguous)
    xtd = nc.dram_tensor("xtd", [HP, TWB], dt.bfloat16)
    ytd = nc.dram_tensor("ytd", [HP, TWB], dt.bfloat16)

    def strided(tile_ap, offset, dims):
        """Custom (possibly overlapping) AP on a 2-D SBUF tile."""
        a = tile_ap.copy()
        a.ap = bass_rust.VecI64Pair([tuple(a.ap[0])] + list(dims))
        a.offset = a.offset + offset
        return a

    with TileContext(nc) as tc:
        with (
            tc.tile_pool(name="io", bufs=1) as io,
            tc.tile_pool(name="feed", bufs=1) as feed,
            tc.tile_pool(name="psA", bufs=1, space="PSUM") as psA,
            tc.tile_pool(name="stage", bufs=1) as stage,
            tc.tile_pool(name="img", bufs=1) as imgp,
            tc.tile_pool(name="psR", bufs=2, space="PSUM") as psR,
        ):
            A = nc.scalar     # ACT engine (+ 2nd HWDGE DMA ring)
            V = nc.vector
            G_ = nc.gpsimd
            S = nc.sync

            wls = io.tile([C, 32], dt.bfloat16)
            wrs = io.tile([C, 32], dt.bfloat16)
            rts = io.tile([H, OHC], dt.bfloat16)
            S.dma_start(out=wls[:, :], in_=wl[:, :])
            A.dma_start(out=wrs[:, :], in_=wr[:, :])
            A.dma_start(out=rts[:, :], in_=rt[:, :])

            # ---- Phase A: tap GEMMs, 4 h-quarters stacked on psum
            # partitions; quarters streamed so PE starts after the first
            # feed DMA. Chunks are whole h-rows so the copyback lands in the
            # 124-wide block layout.
            RCHUNK = [(0, 4), (4, 4), (8, 4), (12, 4), (16, 4), (20, 3)]
            xt4 = stage.tile([128, QROWS * WB], dt.bfloat16)
            yt4 = stage.tile([128, QROWS * WB], dt.bfloat16)
            # zero the 3+3 pad columns of every 124-block
            for t4, eng in ((xt4, V), (yt4, G_)):
                v4 = t4.rearrange("p (r b) -> p r b", b=WB)
                eng.memset(v4[:, :, 0:3], 0.0)
                eng.memset(v4[:, :, 121:124], 0.0)

            psq = [psA.tile([128, 512], dt.float32, name=f"psq{i}", tag=f"ps{i}")
                   for i in range(6)]
            # all feed DMAs up-front so neither half's matmuls starve
            feeds = {}
            for src_d, ring, key in ((xp, S, "x"), (yp, A, "y")):
                for q in range(4):
                    qt = feed.tile([C, QF], dt.bfloat16, tag=f"feed{key}{q}")
                    ring.dma_start(out=qt[:, :],
                                   in_=src_d[:, q * QF:(q + 1) * QF])
                    feeds[(key, q)] = qt
            for src_d, wt, dst4, dram, ring, key in (
                    (xp, wls, xt4, xtd, S, "x"),
                    (yp, wrs, yt4, ytd, A, "y")):
                qtiles = [feeds[(key, q)] for q in range(4)]
                for q in range(4):
                    for ci, (r0, nr) in enumerate(RCHUNK):
                        nn = nr * WP
                        nc.tensor.matmul(
                            psq[ci][q * 32:(q + 1) * 32, :nn],
                            wt[:, :],
                            qtiles[q][:, r0 * WP:r0 * WP + nn],
                            start=True, stop=True,
                            tile_position=(0, q * 32))
                # copybacks after all 4 quarters hit a chunk's psum
                d4 = dst4.rearrange("p (r b) -> p r b", b=WB)
                for ci, (r0, nr) in enumerate(RCHUNK):
                    A.activation(
                        d4[:, r0:r0 + nr, 3:121],
                        psq[ci][:, :nr * WP].rearrange(
                            "p (r w) -> p r w", w=WP),
                        ActF.Copy)
                # write-out: h-major scatter (small descriptors, overlapped)
                vd = dram.rearrange("h (t b) -> t h b", b=WB)
                for q in range(4):
                    rng = ring if q % 2 == 0 else (A if ring is S else S)
                    rng.dma_start(
                        out=vd[:, q * QROWS:(q + 1) * QROWS, :],
                        in_=dst4[q * 32:q * 32 + NTAP, :].rearrange(
                            "p (r b) -> p r b", b=WB))

            # ---- contiguous read-back + two partition-shifted copies
            # (compute engines need start partition 0/32/64/96).
            xtT = stage.tile([HP, TWB], dt.bfloat16)
            ytT = stage.tile([HP, TWB], dt.bfloat16)
            S.dma_start(out=xtT[:, :], in_=xtd[:, :])
            A.dma_start(out=ytT[:, :], in_=ytd[:, :])
            XKH = [xtT]
            YKH = [ytT]
            for kh in (1, 2):
                xk = stage.tile([H, TWB], dt.bfloat16, name=f"xk{kh}")
                yk = stage.tile([H, TWB], dt.bfloat16, name=f"yk{kh}")
                S.dma_start(out=xk[:, :], in_=xtT[kh:kh + H, :])
                A.dma_start(out=yk[:, :], in_=ytT[kh:kh + H, :])
                XKH.append(xk)
                YKH.append(yk)

            def xg(kh, kw):
                """Left term, kd-grouped: [88, 3, 116] (blocks kd=0,1,2)."""
                v = XKH[kh].rearrange("p (t b) -> p t b", b=WB)
                t0 = kh * 3 + kw
                return v[0:H, t0:t0 + 19:9, kw + 3:kw + 119]

            def yg(kh, kw):
                """Right term, kd-grouped, u in [-2,115]: [88, 3, 118].
                Block col = u + kw - kd + 5; kd-block stride = 9*WB - 1."""
                t0 = kh * 3 + kw
                base = t0 * WB + (kw + 2)   # kd=0, u=-2: grid col u+kw+1 -> +3
                return strided(YKH[kh][0:H, :], base,
                               [(9 * WB - 1, 3), (1, 118)])

            # ---- Phase B images, kd-grouped.
            def accw(eng, dst, terms, init_act=True):
                if init_act:
                    A.activation(dst, terms[0], ActF.Copy)
                    rest = terms[1:]
                else:
                    rest = terms
                for t in rest:
                    eng.tensor_tensor(out=dst, in0=dst, in1=t, op=Alu.add)

            # S_kd[j] = sum over kh, kw>=j of term(kd,kh,kw): [88, 3(kd), 116]
            SA0 = imgp.tile([H, 3 * W], dt.bfloat16)
            SA1 = imgp.tile([H, 3 * W], dt.bfloat16)
            SA2 = imgp.tile([H, 3 * W], dt.bfloat16)
            sa0 = SA0.rearrange("p (k w) -> p k w", w=W)
            sa1 = SA1.rearrange("p (k w) -> p k w", w=W)
            sa2 = SA2.rearrange("p (k w) -> p k w", w=W)
            accw(V, sa0[:, :, :], [xg(kh, kw) for kh in range(3)
                                   for kw in range(3)])
            accw(V, sa1[:, :, :], [xg(kh, kw) for kh in range(3)
                                   for kw in (1, 2)])
            accw(V, sa2[:, :, :], [xg(kh, 2) for kh in range(3)])
            # combos; aliases: Gm2 = SA2.k0, P20 = SA2.k2, P21 = SA1.k2
            Fi = imgp.tile([H, W], dt.bfloat16)
            F0 = imgp.tile([H, W], dt.bfloat16)
            F32 = imgp.tile([H, W], dt.bfloat16)
            Gm1 = imgp.tile([H, W], dt.bfloat16)
            G0 = imgp.tile([H, W], dt.bfloat16)
            G1 = imgp.tile([H, W], dt.bfloat16)
            V.tensor_tensor(out=F32[:, :], in0=sa0[:, 0, :], in1=sa0[:, 1, :],
                            op=Alu.add)
            V.tensor_tensor(out=F0[:, :], in0=sa0[:, 1, :], in1=sa0[:, 2, :],
                            op=Alu.add)
            V.tensor_tensor(out=Fi[:, :], in0=F32[:, :], in1=sa0[:, 2, :],
                            op=Alu.add)
            V.tensor_tensor(out=Gm1[:, :], in0=sa1[:, 0, :], in1=sa2[:, 1, :],
                            op=Alu.add)
            V.tensor_tensor(out=G0[:, :], in0=sa0[:, 0, :], in1=sa1[:, 1, :],
                            op=Alu.add)
            V.tensor_tensor(out=G0[:, :], in0=G0[:, :], in1=sa2[:, 2, :],
                            op=Alu.add)
            V.tensor_tensor(out=G1[:, :], in0=F32[:, :], in1=sa1[:, 2, :],
                            op=Alu.add)
            Gm2 = sa2[:, 0, :]
            P20 = sa2[:, 2, :]
            P21 = sa1[:, 2, :]
            # edge columns
            Gcol0 = imgp.tile([H, 2], dt.bfloat16)    # d=0: w=0,1
            Gcol32 = imgp.tile([H, 4], dt.bfloat16)   # d=32: w=30..33
            V.tensor_tensor(out=Gcol0[:, 0:1], in0=G0[:, 0:1],
                            in1=sa0[:, 0, 0:1], op=Alu.subtract)
            V.tensor_tensor(out=Gcol0[:, 1:2], in0=G1[:, 1:2],
                            in1=sa0[:, 0, 1:2], op=Alu.subtract)
            A.activation(Gcol32[:, 0:1], Gm2[:, 30:31], ActF.Copy)
            A.activation(Gcol32[:, 1:2], Gm1[:, 31:32], ActF.Copy)
            V.tensor_tensor(out=Gcol32[:, 2:3], in0=G0[:, 32:33],
                            in1=P20[:, 32:33], op=Alu.subtract)
            V.tensor_tensor(out=Gcol32[:, 3:4], in0=G1[:, 33:34],
                            in1=P21[:, 33:34], op=Alu.subtract)

            # Right-half images: RKc [88, 3(kd), 150], col = u+32, u in
            # [-2,115] -> cols 30..147; cols 0..29,148..149 stay zero.
            RKc = imgp.tile([H, 3 * 150], dt.bfloat16)
            RCc = imgp.tile([H, 3 * 150], dt.bfloat16)
            rkc = RKc.rearrange("p (k u) -> p k u", u=150)
            rcc = RCc.rearrange("p (k u) -> p k u", u=150)
            G_.memset(RKc[:, :], 0.0)
            V.memset(RCc[:, :], 0.0)
            for kh in range(3):
                for kw in range(3):
                    G_.tensor_tensor(out=rkc[:, :, 30:148],
                                     in0=rkc[:, :, 30:148],
                                     in1=yg(kh, kw), op=Alu.add)
                V.tensor_tensor(out=rcc[:, :, 30:148],
                                in0=rcc[:, :, 30:148],
                                in1=yg(kh, 2), op=Alu.add)
            Ri = imgp.tile([H, 150], dt.bfloat16)
            R0 = imgp.tile([H, 150], dt.bfloat16)
            R32 = imgp.tile([H, 150], dt.bfloat16)
            Rcorr = imgp.tile([H, 150], dt.bfloat16)
            G_.tensor_tensor(out=R32[:, :], in0=rkc[:, 0, :], in1=rkc[:, 1, :],
                             op=Alu.add)
            G_.tensor_tensor(out=R0[:, :], in0=rkc[:, 1, :], in1=rkc[:, 2, :],
                             op=Alu.add)
            G_.tensor_tensor(out=Ri[:, :], in0=R32[:, :], in1=rkc[:, 2, :],
                             op=Alu.add)
            V.tensor_tensor(out=Rcorr[:, :], in0=rcc[:, 0, :],
                            in1=rcc[:, 1, :], op=Alu.add)
            V.tensor_tensor(out=Rcorr[:, :], in0=Rcorr[:, :],
                            in1=rcc[:, 2, :], op=Alu.add)
            Rc0 = imgp.tile([H, 1], dt.bfloat16)   # d=0 (u=115): kd in {1,2}
            Rc32 = imgp.tile([H, 1], dt.bfloat16)  # d=32 (u=83): kd in {0,1}
            G_.tensor_tensor(out=Rc0[:, :], in0=rcc[:, 1, 147:148],
                             in1=rcc[:, 2, 147:148], op=Alu.add)
            G_.tensor_tensor(out=Rc32[:, :], in0=rcc[:, 0, 115:116],
                             in1=rcc[:, 1, 115:116], op=Alu.add)

            # ---- Assembly: cost [88, 33*116] bf16
            cost = stage.tile([H, D * W], dt.bfloat16)
            costv = cost.rearrange("p (d w) -> p d w", w=W)
            # 1. F select over all d (w - d - 2 >= 0 keep else 0)
            G_.affine_select(
                out=costv[:, :, :],
                in_=Fi[:, :].unsqueeze(1).broadcast_to((H, D, W)),
                pattern=[[-1, D], [1, W]], base=-2,
                compare_op=Alu.is_ge, fill=0.0, channel_multiplier=0)
            # 2. G diagonal writes (interior d), on ACT (1-input copies)
            for tp, dlo, img in ((-2, 2, Gm2), (-1, 1, Gm1[:, :]),
                                 (0, 1, G0[:, :]), (1, 1, G1[:, :])):
                cnt = 31 - dlo + 1
                s = dlo * 117 + tp
                A.activation(cost[:, s:s + 117 * cnt:117],
                             img[:, dlo + tp:dlo + tp + cnt], ActF.Copy)
            # 3. R diagonal add over all d: cost[:,d,w] += R[:, 32+w-d]
            DSPL = 17
            V.tensor_tensor(
                out=costv[:, :DSPL, :], in0=costv[:, :DSPL, :],
                in1=strided(Ri[:, :], 32, [(-1, DSPL), (1, W)]), op=Alu.add)
            G_.tensor_tensor(
                out=costv[:, DSPL:, :], in0=costv[:, DSPL:, :],
                in1=strided(Ri[:, :], 32 - DSPL, [(-1, D - DSPL), (1, W)]),
                op=Alu.add)
            # 4. right-edge corr (interior d): cost[:,d,115] -= Rcorr[:,147-d]
            V.tensor_tensor(out=cost[:, 231:231 + 116 * 31:116],
                            in0=cost[:, 231:231 + 116 * 31:116],
                            in1=Rcorr[:, 146:115:-1], op=Alu.subtract)
            # 5. fixup d=0
            G_.affine_select(out=costv[:, 0, :], in_=F0[:, :],
                             pattern=[[1, W]], base=-2,
                             compare_op=Alu.is_ge, fill=0.0,
                             channel_multiplier=0)
            V.tensor_copy(out=cost[:, 0:2], in_=Gcol0[:, :])
            V.tensor_tensor(out=costv[:, 0, :], in0=costv[:, 0, :],
                            in1=R0[:, 32:148], op=Alu.add)
            V.tensor_tensor(out=cost[:, 115:116], in0=cost[:, 115:116],
                            in1=Rc0[:, :], op=Alu.subtract)
            # 6. fixup d=32
            G_.affine_select(out=costv[:, 32, :], in_=F32[:, :],
                             pattern=[[1, W]], base=-34,
                             compare_op=Alu.is_ge, fill=0.0,
                             channel_multiplier=0)
            V.tensor_copy(out=cost[:, 32 * 116 + 30:32 * 116 + 34],
                          in_=Gcol32[:, :])
            V.tensor_tensor(out=costv[:, 32, :], in0=costv[:, 32, :],
                            in1=R32[:, 0:116], op=Alu.add)
            V.tensor_tensor(out=cost[:, 32 * 116 + 115:32 * 116 + 116],
                            in0=cost[:, 32 * 116 + 115:32 * 116 + 116],
                            in1=Rc32[:, :], op=Alu.subtract)

            # ---- Resize: row matmul; column phases contiguous (p0,p1,p2),
            # interleaved on the host. Work split in two d-halves so the
            # first half's output DMAs overlap the second half's compute.
            u = stage.tile([OHC, D * W], dt.bfloat16)
            v = stage.tile([OHC, D * W], dt.bfloat16)
            p0 = stage.tile([OHC, D * W], dt.float32)
            p1 = stage.tile([OHC, D * W], dt.float32)
            p2 = stage.tile([OHC, D * W], dt.float32)
            uvv = u.rearrange("p (d w) -> p d w", w=W)
            vvv = v.rearrange("p (d w) -> p d w", w=W)
            p1v = p1.rearrange("p (d w) -> p d w", w=W)
            p2v = p2.rearrange("p (d w) -> p d w", w=W)
            HALVES = [(0, DSPL, (0, 512), (512, 512), (1024, 512),
                       (1536, 460)),
                      (DSPL, D, (1972, 512), (2484, 512), (2996, 512),
                       (3508, 320))]
            for dlo, dhi, *chunks in HALVES:
                for off, nn in chunks:
                    ps = psR.tile([OHC, 512], dt.float32, tag="psR")
                    nc.tensor.matmul(ps[:, :nn], rts[:, :],
                                     cost[:, off:off + nn],
                                     start=True, stop=True)
                    A.activation(u[:, off:off + nn], ps[:, :nn], ActF.Copy,
                                 scale=1.0 / 3.0)
                    A.activation(v[:, off:off + nn], ps[:, :nn], ActF.Copy,
                                 scale=2.0 / 3.0)
                c0, c1 = dlo * W, dhi * W
                # p0 = u + v (= r1); p1[m] = v[m]+u[m+1]; p2[m] = u[m]+v[m+1]
                V.tensor_tensor(out=p0[:, c0:c1], in0=u[:, c0:c1],
                                in1=v[:, c0:c1], op=Alu.add)
                V.tensor_tensor(out=p1v[:, dlo:dhi, 0:115],
                                in0=vvv[:, dlo:dhi, 0:115],
                                in1=uvv[:, dlo:dhi, 1:116], op=Alu.add)
                G_.tensor_tensor(out=p2v[:, dlo:dhi, 0:115],
                                 in0=uvv[:, dlo:dhi, 0:115],
                                 in1=vvv[:, dlo:dhi, 1:116], op=Alu.add)
                for r, (pt, ring) in enumerate(((p0, S), (p1, A), (p2, S))):
                    wlim = W if r == 0 else W - 1
                    ring.dma_start(
                        out=outd[r, dlo:dhi].rearrange(
                            "d j w -> j d w")[:, :, 0:wlim],
                        in_=pt.rearrange(
                            "p (d w) -> p d w", w=W)[:, dlo:dhi, 0:wlim])
    nc.compile()
    return nc


def _prep_inputs(x_feat, y_feat, w_match):
    """Host-side shard prep: per-core input dicts."""
    x_feat = np.asarray(x_feat, dtype=np.float32)
    y_feat = np.asarray(y_feat, dtype=np.float32)
    w_match = np.asarray(w_match, dtype=np.float32)
    wl = np.zeros((C, 32), dtype=BF16)
    wr = np.zeros((C, 32), dtype=BF16)
    wl[:, :NTAP] = w_match[0, :C].reshape(C, NTAP)
    wr[:, :NTAP] = w_match[0, C:].reshape(C, NTAP)
    Rt = _row_matrix()
    in_maps = []
    for core in range(8):
        n, q = divmod(core, 4)
        xpad = np.zeros((C, HP, WP), dtype=BF16)
        ypad = np.zeros((C, HP, WP), dtype=BF16)
        xpad[:, 1:89, 1:117] = x_feat[n, 2]
        ypad[:, 1:89, 1:117] = y_feat[n, 2]
        in_maps.append({
            "xp": xpad.reshape(C, FLAT),
            "yp": ypad.reshape(C, FLAT),
            "wl": wl, "wr": wr,
            "rt": Rt[:, q * OHC:(q + 1) * OHC].astype(BF16),
        })
    return in_maps


def _interleave(out_slice, ph):
    """ph: [3, 33, 65, 116] phase-major -> out_slice [33, 65, 346]."""
    out_slice[:, :, 0::3] = ph[0]
    out_slice[:, :, 1::3] = ph[1][:, :, :115]
    out_slice[:, :, 2::3] = ph[2][:, :, :115]


def kernel(x_feat, y_feat, w_match):
    from concourse.bass_utils import run_bass_kernel_spmd

    if "nc" not in _BUILT:
        _BUILT["nc"] = _build_nc()
    nc = _BUILT["nc"]
    in_maps = _prep_inputs(x_feat, y_feat, w_match)
    trace = bool(int(os.environ.get("KERNEL_TRACE", "0")))
    res = run_bass_kernel_spmd(
        nc, in_maps, core_ids=list(range(8)),
        trace=trace,
        trace_cores=list(range(8)) if trace else None,
    )
    _BUILT["last_result"] = res
    out = np.empty((2, D, OH, OW), dtype=np.float32)
    for core in range(8):
        n, q = divmod(core, 4)
        _interleave(out[n, :, q * OHC:(q + 1) * OHC, :],
                    res.results[core]["out"])
    return out
